# revision 31
# baseline (speedup 1.0000x reference)
"""CRF loss kernel for Trainium2 (8 NeuronCores, sequence-parallel).

reference: mean_b( logZ_b - score_b ) for a linear-chain CRF with
B=256, S=512, T=128.

The forward algorithm's 511-step recurrence u_s = (A^T u_{s-1}) o e_s
is latency-bound on device (~0.9 us per step of matmul + DVE multiply
incl. PSUM drain/ack and semaphores), so instead of data-parallel batch
sharding this kernel shards the SEQUENCE: products of positive matrices
forget their starting direction at ~e^-1.9/step (measured: L1 direction
error 2.9e-4 after 4 steps), so each core computes one ~64-step segment
for ALL 256 batches, warm-starting its incoming state with a 4-step
warmup, and the host telescopes

    logZ_b = sum_c log r_cb - sum_{c>=1} log n_cb + 511*kappa

where r_c = 1^T M_seg_c v~_c (bilinear segment value, computed
meet-in-the-middle with an exact ones/end-vector backward chain) and
n_c = 1^T v~_c normalizes the warmup state at the segment boundary.
Serial depth per core: 34 rounds (vs 256 for batch-parallel
meet-in-the-middle), each round = fwd step + bwd step at width 256,
anti-phased so TensorE and VectorE ping-pong between the two chains.

kappa (exact per-step log growth of batch 0, host fp64) is
pre-subtracted from the emissions so everything stays O(e^+-5) in bf16.
Emissions are pre-exponentiated on the host; segment-length differences
between cores (core 0 has no warmup, core 7 a 63-step segment) are
handled by per-core no-op pad steps: identity stationary slots in the
per-core boot inputs plus all-ones emission tiles, so a single uniform
program runs on all 8 cores. Emission feed uses two parallel DMA
trigger queues (Sync for fwd, GpSimd for bwd).

Numerator (score of the tagged path, ~0.1% of FLOPs) on host in fp64.
"""

import numpy as np
import ml_dtypes

B, S, T = 256, 512, 128
NCORES = 8
NF = NB = 33               # fwd/bwd slots per core
TAPK = 1                   # n_c tap after fwd slot index 1 (2 slots)

_nc_cache = None
LAST_RESULTS = None


def _slot_plan(c):
    """Per-core (fwd_steps, bwd_steps); None = pad (identity step).

    Segments: core 0: steps 1..64, cores 1..6: 64c+1..64c+64,
    core 7: 449..511. Warmup: 2 steps before the segment (1 + one pad
    for core 7; none for core 0). Meet after fwd's last step.
    """
    if c == 0:
        f = list(range(1, 34))                      # 33 real, no warmup
        b = [None] * 2 + list(range(64, 33, -1))    # 2 pads + 64..34
    elif c < 7:
        f = list(range(64 * c - 1, 64 * c + 32))    # 2 warm + 31 seg
        b = list(range(64 * c + 64, 64 * c + 31, -1))  # 33 real
    else:
        f = [None] + list(range(448, 480))          # pad + 1 warm + 31 seg
        b = [None] + list(range(511, 479, -1))      # pad + 511..480
    assert len(f) == NF and len(b) == NB
    return f, b


def _build_nc():
    import concourse.bacc as bacc
    import concourse.mybir as mybir
    import concourse.tile as tile

    fp32 = mybir.dt.float32
    bf16 = mybir.dt.bfloat16
    mult = mybir.AluOpType.mult

    nc = bacc.Bacc("TRN2", target_bir_lowering=False, debug=False)

    LEAD = 1               # fwd/bwd slot 0 rides in the boot DMAs
    emf = nc.dram_tensor("emf", [T, NF - LEAD, B], bf16, kind="ExternalInput")
    emb = nc.dram_tensor("emb", [T, NB - LEAD, B], bf16, kind="ExternalInput")
    # bootf = [F_X | A | finit | emf slot 0], bootb likewise for bwd
    bootf = nc.dram_tensor("bootf", [T, 2 * T + (1 + LEAD) * B], bf16,
                           kind="ExternalInput")
    bootb = nc.dram_tensor("bootb", [T, 2 * T + (1 + LEAD) * B], bf16,
                           kind="ExternalInput")
    atp = nc.dram_tensor("atp", [T, T], bf16, kind="ExternalInput")
    outr = nc.dram_tensor("outr", [T, B], bf16, kind="ExternalOutput")
    outn = nc.dram_tensor("outn", [T, B], bf16, kind="ExternalOutput")

    chunks = [(1, 2), (3, 5), (8, 6), (14, 6), (20, 6), (26, 7)]

    with tile.TileContext(nc) as tc:
        with (
            tc.tile_pool(name="const", bufs=1) as constp,
            tc.tile_pool(name="emp", bufs=2 * len(chunks)) as emp,
            tc.tile_pool(name="sbp", bufs=4) as sbp,
            tc.tile_pool(name="vp", bufs=2, space="PSUM") as vp,
            tc.tile_pool(name="bp", bufs=2, space="PSUM") as bp,

        ):
            bf_tile = constp.tile([T, 2 * T + (1 + LEAD) * B], bf16)
            bb_tile = constp.tile([T, 2 * T + (1 + LEAD) * B], bf16)
            at_tile = constp.tile([T, T], bf16)

            fx_ap = bf_tile[:, 0:T]
            fa_ap = bf_tile[:, T:2 * T]
            fi_ap = bf_tile[:, 2 * T:2 * T + B]
            bx_ap = bb_tile[:, 0:T]
            by_ap = bb_tile[:, T:2 * T]
            bi_ap = bb_tile[:, 2 * T:2 * T + B]

            emf_tiles = {}
            emb_tiles = {}

            def load_chunk(ci, which="fb"):
                s0, ln = chunks[ci]
                if "f" in which:
                    tf = emp.tile([T, ln, B], bf16, tag="emf")
                    nc.sync.dma_start(tf[:], emf[:, s0 - LEAD:s0 - LEAD + ln, :])
                    emf_tiles[ci] = tf
                if "b" in which:
                    tb = emp.tile([T, ln, B], bf16, tag="emb")
                    nc.scalar.dma_start(tb[:], emb[:, s0 - LEAD:s0 - LEAD + ln, :])
                    emb_tiles[ci] = tb

            def em_slice(boot_tile, tiles, k):
                if k < LEAD:
                    a = 2 * T + (1 + k) * B
                    return boot_tile[:, a:a + B]
                for ci, (s0, ln) in enumerate(chunks):
                    if s0 <= k < s0 + ln:
                        return tiles[ci][:, k - s0, :]
                raise AssertionError(k)

            # fwd feed on the Sync DMA queue, bwd feed on the Scalar queue
            nc.sync.dma_start(bf_tile[:], bootf[:])
            nc.scalar.dma_start(bb_tile[:], bootb[:])
            load_chunk(0)
            nc.scalar.dma_start(at_tile[:], atp[:])
            for ci in range(1, len(chunks)):
                load_chunk(ci)

            def fstat(k):
                return fx_ap if k == 0 else fa_ap

            def bstat(k):
                return bx_ap if k == 0 else (by_ap if k <= TAPK else at_tile[:])

            u = fi_ap
            z_prev = bi_ap          # bwd state (SBUF AP before slot 0)
            zp = None
            for k in range(NF):
                # fwd slot k: u <- (stat^T u) o e_f[k]
                vf = vp.tile([T, B], fp32, tag="vf")
                nc.tensor.matmul(vf[:], fstat(k), u, start=True, stop=True)
                u_new = sbp.tile([T, B], bf16, tag="u")
                nc.vector.tensor_tensor(u_new[:], vf[:],
                                        em_slice(bf_tile, emf_tiles, k), mult)
                u = u_new[:]
                # bwd slot k: z <- stat^T (e_b[k] o z)
                tmp = sbp.tile([T, B], bf16, tag="w")
                zsrc = z_prev if zp is None else zp[:]
                nc.vector.tensor_tensor(tmp[:], zsrc,
                                        em_slice(bb_tile, emb_tiles, k), mult)
                zp = bp.tile([T, B], fp32, tag="vb")
                nc.tensor.matmul(zp[:], bstat(k), tmp[:], start=True, stop=True)
                if k == TAPK:
                    # n_c tap: ship the post-warmup state, host reduces
                    nc.gpsimd.dma_start(outn[:], u)

            # meet: r_c = sum_t z[t,b] * u[t,b]; host does the t-reduction
            prod = sbp.tile([T, B], bf16, tag="u")
            nc.vector.tensor_tensor(prod[:], zp[:], u, mult)
            nc.sync.dma_start(outr[:], prod[:])

    nc.compile()
    return nc


def _get_nc():
    global _nc_cache
    if _nc_cache is None:
        _nc_cache = _build_nc()
    return _nc_cache


def _ensure_ntff_hook_importable():
    """bass_utils imports antenv.axon_hooks when BASS_TRACE is set; this
    image's antenv package lacks that module, so provide a shim rather
    than crash (and enable profiling when the axon .so supports it)."""
    import sys
    import types
    try:
        import antenv.axon_hooks  # noqa: F401
        return
    except ImportError:
        pass
    try:
        import antenv
        from trn_agent_boot.trn_boot import _ntff_profile_via_ctypes
        hook = _ntff_profile_via_ctypes('/opt/axon/libaxon_pjrt.so')
    except Exception:
        try:
            import antenv
        except ImportError:
            return
        hook = None
    mod = types.ModuleType("antenv.axon_hooks")
    mod._hook = hook
    mod.get_axon_ntff_profile_hook = lambda: mod._hook
    mod.set_axon_ntff_profile_hook = lambda h: setattr(mod, "_hook", h)
    antenv.axon_hooks = mod
    sys.modules["antenv.axon_hooks"] = mod


def _kappa_host(em, trans, start):
    """Exact per-step log-mass growth of batch 0 (fp64 log-space forward)."""
    sc = start.astype(np.float64) + em[0, 0].astype(np.float64)
    t64 = trans.astype(np.float64)
    for i in range(1, em.shape[1]):
        x = sc[:, None] + t64 + em[0, i].astype(np.float64)[None, :]
        mx = x.max(axis=0)
        sc = mx + np.log(np.exp(x - mx[None, :]).sum(axis=0))
    mx = sc.max()
    return float((mx + np.log(np.exp(sc - mx).sum())) / (em.shape[1] - 1))


def _numerator_host(em, tags, mask, trans, start, end):
    em64 = em.astype(np.float64)
    tags = tags.astype(np.int64)
    bidx = np.arange(em.shape[0])
    score = start.astype(np.float64)[tags[:, 0]] + em64[bidx, 0, tags[:, 0]]
    trans_term = trans.astype(np.float64)[tags[:, 1:], tags[:, :-1]]
    em_term = np.take_along_axis(em64[:, 1:], tags[:, 1:, None], axis=2)[..., 0]
    m = mask[:, 1:].astype(np.float64)
    score = score + ((trans_term + em_term) * m).sum(axis=1)
    last_idx = mask.sum(axis=1).astype(np.int64) - 1
    last_tags = np.take_along_axis(tags, last_idx[:, None], axis=1)[:, 0]
    return score + end.astype(np.float64)[last_tags]


def _reference_host(em, tags, mask, trans, start, end):
    """Pure-numpy fp64 fallback (exact semantics incl. arbitrary masks)."""
    em64 = em.astype(np.float64)
    score = start.astype(np.float64) + em64[:, 0]  # [B, T]
    t64 = trans.astype(np.float64)
    for i in range(1, em.shape[1]):
        x = score[:, :, None] + t64[None] + em64[:, i][:, None, :]
        mx = x.max(axis=1)
        nxt = mx + np.log(np.exp(x - mx[:, None, :]).sum(axis=1))
        score = np.where(mask[:, i][:, None], nxt, score)
    x = score + end.astype(np.float64)
    mx = x.max(axis=1, keepdims=True)
    denom = (mx[:, 0] + np.log(np.exp(x - mx).sum(axis=1)))
    numer = _numerator_host(em, tags, mask, trans, start, end)
    return np.float32((denom - numer).mean())


def kernel(**inputs):
    global LAST_RESULTS
    em = np.asarray(inputs["emissions"], dtype=np.float32)
    tags = np.asarray(inputs["tags"])
    mask = np.asarray(inputs["mask"])
    trans = np.asarray(inputs["transitions"], dtype=np.float32)
    start = np.asarray(inputs["start_transitions"], dtype=np.float32)
    end = np.asarray(inputs["end_transitions"], dtype=np.float32)

    if not mask.all():
        # device scan assumes a dense mask (guaranteed by the input spec);
        # fall back to the exact host path otherwise
        return _reference_host(em, tags, mask, trans, start, end)

    _ensure_ntff_hook_importable()
    from concourse.bass_utils import run_bass_kernel_spmd

    nc = _get_nc()
    kap = _kappa_host(em, trans, start)
    bf = ml_dtypes.bfloat16

    # exp-space, kappa-scaled emission multipliers, [T, S, B]
    ex = np.exp(em.transpose(2, 1, 0) - np.float32(kap)).astype(bf)
    ones_tb = np.ones((T, B), dtype=bf)
    a_exp = np.exp(trans).astype(bf)
    at_exp = np.ascontiguousarray(a_exp.T)
    id_t = np.eye(T, dtype=bf)
    u0 = np.ascontiguousarray(
        np.exp(start[None, :] + em[:, 0, :]).T.astype(bf))      # [T, B]
    endv = np.ascontiguousarray(
        np.tile(np.exp(end).astype(bf)[:, None], (1, B)))

    LEAD = 1

    def gather(steps):
        cols = [ones_tb[:, None, :] if s is None else ex[:, s:s + 1, :]
                for s in steps]
        return np.ascontiguousarray(np.concatenate(cols, axis=1))

    def flat(steps):
        cols = [ones_tb if s is None else ex[:, s, :] for s in steps]
        return np.concatenate(cols, axis=1)

    in_maps = []
    spans_r = np.zeros(NCORES)
    spans_n = np.zeros(NCORES)
    for c in range(NCORES):
        f, b = _slot_plan(c)
        spans_r[c] = sum(s is not None for s in f) + sum(s is not None for s in b)
        spans_n[c] = sum(s is not None for s in f[:TAPK + 1])
        fx = id_t if c == 7 else a_exp
        bx = id_t if c in (0, 7) else at_exp
        by = id_t if c == 0 else at_exp
        in_maps.append({
            "emf": gather(f[LEAD:]),
            "emb": gather(b[LEAD:]),
            "bootf": np.ascontiguousarray(np.concatenate(
                [fx, a_exp, u0 if c == 0 else ones_tb, flat(f[:LEAD])], axis=1)),
            "bootb": np.ascontiguousarray(np.concatenate(
                [bx, by, endv if c == 7 else ones_tb, flat(b[:LEAD])], axis=1)),
            "atp": at_exp,
        })

    LAST_RESULTS = run_bass_kernel_spmd(nc, in_maps, list(range(NCORES)))
    r = np.stack([LAST_RESULTS.results[c]["outr"] for c in range(NCORES)])
    n_tap = np.stack([LAST_RESULTS.results[c]["outn"] for c in range(NCORES)])
    r = r.astype(np.float64).sum(axis=1)    # [8, B]
    n = n_tap.astype(np.float64).sum(axis=1)

    ok = np.isfinite(r).all() and (r > 0).all()
    ok = ok and np.isfinite(n[1:]).all() and (n[1:] > 0).all()
    if not ok:
        return _reference_host(em, tags, mask, trans, start, end)

    logz = (np.log(r).sum(axis=0) - np.log(n[1:]).sum(axis=0)
            + kap * (spans_r.sum() - spans_n[1:].sum()))
    numer = _numerator_host(em, tags, mask, trans, start, end)
    return np.float32((logz - numer).mean())


# revision 32
# speedup vs baseline: 1.0616x; 1.0616x over previous
"""CRF loss kernel for Trainium2 (8 NeuronCores, sequence-parallel).

reference: mean_b( logZ_b - score_b ) for a linear-chain CRF with
B=256, S=512, T=128.

The forward algorithm's 511-step recurrence u_s = (A^T u_{s-1}) o e_s
is latency-bound on device (~0.9 us per step of matmul + DVE multiply
incl. PSUM drain/ack and semaphores), so instead of data-parallel batch
sharding this kernel shards the SEQUENCE: products of positive matrices
forget their starting direction at ~e^-1.9/step (measured), so each
core computes one ~64-step segment for ALL 256 batches, warm-starting
its incoming state with a 2-step warmup, and the host telescopes

    logZ_b = sum_c log r_cb - sum_{c>=1} log n_cb + 511*cs

where r_c = 1^T M_seg_c v~_c (bilinear segment value, computed
meet-in-the-middle with an exact ones/end-vector backward chain),
n_c = 1^T v~_c normalizes the warmup state at the segment boundary, and
cs is a per-step log rescale (from an exact host fp64 forward of batch
0) that keeps everything in range. Serial depth per core: 33 rounds (vs
256 for batch-parallel meet-in-the-middle), each round = fwd step + bwd
step at width 256, anti-phased so TensorE and VectorE ping-pong.

Emissions are pre-exponentiated on the host and shipped as fp8 e4m3
multipliers (64 KB per round vs the ~138 B/ns single-queue DMA feed —
bf16 would be feed-bound; fp8 costs ~2e-1 absolute on a ~3000 logZ,
1e-4 relative). Both chains start from an on-device memset ones state;
per-core inits (u_0 for core 0, exp(end) for core 7) and per-core
segment-length pads ride in the slot-0 emissions with per-core identity
stationaries, so a single uniform program runs on all 8 cores.

Numerator (score of the tagged path, ~0.1% of FLOPs) on host in fp64.
"""

import numpy as np
import ml_dtypes

B, S, T = 256, 512, 128
NCORES = 8
NF = NB = 33               # fwd/bwd slots per core
TAPK = 1                   # n_c tap after fwd slot index 1

_nc_cache = None
LAST_RESULTS = None


def _plans(c):
    """Per-core fwd/bwd slot lists: (stationary, emission_spec).

    stationary: 'A' (=exp(trans), fwd), 'T' (=exp(trans).T, bwd),
    'I' (identity pad). emission_spec: ('step', s) | ('ones',) |
    ('u0',) | ('end',).
    Segments: core 0: steps 1..64, cores 1..6: 64c+1..64c+64,
    core 7: 449..511. Slot 0 doubles as the init loader: with an
    identity stationary and a ones init state, u after slot 0 equals
    the slot-0 "emission" tile.
    """
    if c == 0:
        f = [('I', ('u0',))] + [('A', ('step', s)) for s in range(1, 33)]
        b = [('I', ('ones',))] + [('T', ('step', s))
                                  for s in range(64, 32, -1)]
    elif c < 7:
        f = ([('A', ('step', 64 * c - 1)), ('A', ('step', 64 * c))]
             + [('A', ('step', s)) for s in range(64 * c + 1, 64 * c + 32)])
        b = [('T', ('step', s)) for s in range(64 * c + 64, 64 * c + 31, -1)]
    else:
        f = ([('I', ('ones',)), ('A', ('step', 448))]
             + [('A', ('step', s)) for s in range(449, 480)])
        b = [('I', ('end',))] + [('T', ('step', s))
                                 for s in range(511, 479, -1)]
    assert len(f) == NF and len(b) == NB
    return f, b


def _build_nc():
    import concourse.bacc as bacc
    import concourse.mybir as mybir
    import concourse.tile as tile

    fp32 = mybir.dt.float32
    bf16 = mybir.dt.bfloat16
    f8 = mybir.dt.float8e4
    mult = mybir.AluOpType.mult

    nc = bacc.Bacc("TRN2", target_bir_lowering=False, debug=False)

    emf = nc.dram_tensor("emf", [T, NF, B], f8, kind="ExternalInput")
    emb = nc.dram_tensor("emb", [T, NB, B], f8, kind="ExternalInput")
    bootf = nc.dram_tensor("bootf", [T, 2 * T], bf16, kind="ExternalInput")
    bootb = nc.dram_tensor("bootb", [T, 2 * T], bf16, kind="ExternalInput")
    outr = nc.dram_tensor("outr", [T, B], bf16, kind="ExternalOutput")
    outn = nc.dram_tensor("outn", [T, B], bf16, kind="ExternalOutput")

    chunks = [(0, 2), (2, 4), (6, 6), (12, 7), (19, 7), (26, 7)]

    with tile.TileContext(nc) as tc:
        with (
            tc.tile_pool(name="const", bufs=1) as constp,
            tc.tile_pool(name="emp", bufs=2 * len(chunks)) as emp,
            tc.tile_pool(name="sbp", bufs=4) as sbp,
            tc.tile_pool(name="vp", bufs=2, space="PSUM") as vp,
            tc.tile_pool(name="bp", bufs=2, space="PSUM") as bp,
        ):
            bf_tile = constp.tile([T, 2 * T], bf16)
            bb_tile = constp.tile([T, 2 * T], bf16)
            ones_sb = constp.tile([T, B], bf16)
            nc.gpsimd.memset(ones_sb[:], 1.0)

            emf_tiles = {}
            emb_tiles = {}

            def load_chunk(ci, which="fb"):
                s0, ln = chunks[ci]
                if "f" in which:
                    tf = emp.tile([T, ln, B], f8, tag="emf")
                    nc.sync.dma_start(tf[:], emf[:, s0:s0 + ln, :])
                    emf_tiles[ci] = tf
                if "b" in which:
                    tb = emp.tile([T, ln, B], f8, tag="emb")
                    nc.sync.dma_start(tb[:], emb[:, s0:s0 + ln, :])
                    emb_tiles[ci] = tb

            def em_slice(tiles, k):
                for ci, (s0, ln) in enumerate(chunks):
                    if s0 <= k < s0 + ln:
                        return tiles[ci][:, k - s0, :]
                raise AssertionError(k)

            # feed order (one hw DMA queue): fwd boot + lead, then bwd
            nc.sync.dma_start(bf_tile[:], bootf[:])
            load_chunk(0, "f")
            nc.sync.dma_start(bb_tile[:], bootb[:])
            load_chunk(0, "b")
            for ci in range(1, len(chunks)):
                load_chunk(ci)

            fx_ap = bf_tile[:, 0:T]
            fa_ap = bf_tile[:, T:2 * T]
            bx_ap = bb_tile[:, 0:T]
            bat_ap = bb_tile[:, T:2 * T]

            u = ones_sb[:]
            z_prev = ones_sb[:]
            zp = None
            for k in range(NF):
                # fwd slot k: u <- (stat^T u) o e_f[k]
                vf = vp.tile([T, B], fp32, tag="vf")
                nc.tensor.matmul(vf[:], fx_ap if k == 0 else fa_ap, u,
                                 start=True, stop=True)
                u_new = sbp.tile([T, B], bf16, tag="u")
                nc.vector.tensor_tensor(u_new[:], vf[:],
                                        em_slice(emf_tiles, k), mult)
                u = u_new[:]
                # bwd slot k: z <- stat^T (e_b[k] o z)
                tmp = sbp.tile([T, B], bf16, tag="w")
                zsrc = z_prev if zp is None else zp[:]
                nc.vector.tensor_tensor(tmp[:], zsrc,
                                        em_slice(emb_tiles, k), mult)
                zp = bp.tile([T, B], fp32, tag="vb")
                nc.tensor.matmul(zp[:], bx_ap if k == 0 else bat_ap, tmp[:],
                                 start=True, stop=True)
                if k == TAPK:
                    # n_c tap: ship the post-warmup state, host reduces
                    nc.gpsimd.dma_start(outn[:], u)

            # meet: r_c = sum_t z[t,b] * u[t,b]; host does the t-reduction
            prod = sbp.tile([T, B], bf16, tag="u")
            nc.vector.tensor_tensor(prod[:], zp[:], u, mult)
            nc.sync.dma_start(outr[:], prod[:])

    nc.compile()
    return nc


def _get_nc():
    global _nc_cache
    if _nc_cache is None:
        _nc_cache = _build_nc()
    return _nc_cache


def _ensure_ntff_hook_importable():
    """bass_utils imports antenv.axon_hooks when BASS_TRACE is set; this
    image's antenv package lacks that module, so provide a shim rather
    than crash (and enable profiling when the axon .so supports it)."""
    import sys
    import types
    try:
        import antenv.axon_hooks  # noqa: F401
        return
    except ImportError:
        pass
    try:
        import antenv
        from trn_agent_boot.trn_boot import _ntff_profile_via_ctypes
        hook = _ntff_profile_via_ctypes('/opt/axon/libaxon_pjrt.so')
    except Exception:
        try:
            import antenv
        except ImportError:
            return
        hook = None
    mod = types.ModuleType("antenv.axon_hooks")
    mod._hook = hook
    mod.get_axon_ntff_profile_hook = lambda: mod._hook
    mod.set_axon_ntff_profile_hook = lambda h: setattr(mod, "_hook", h)
    antenv.axon_hooks = mod
    sys.modules["antenv.axon_hooks"] = mod


def _kappa_host(em, trans, start):
    """Exact per-step log-mass growth of batch 0 (fp64 log-space forward)."""
    sc = start.astype(np.float64) + em[0, 0].astype(np.float64)
    t64 = trans.astype(np.float64)
    for i in range(1, em.shape[1]):
        x = sc[:, None] + t64 + em[0, i].astype(np.float64)[None, :]
        mx = x.max(axis=0)
        sc = mx + np.log(np.exp(x - mx[None, :]).sum(axis=0))
    mx = sc.max()
    return float((mx + np.log(np.exp(sc - mx).sum())) / (em.shape[1] - 1))


def _numerator_host(em, tags, mask, trans, start, end):
    em64 = em.astype(np.float64)
    tags = tags.astype(np.int64)
    bidx = np.arange(em.shape[0])
    score = start.astype(np.float64)[tags[:, 0]] + em64[bidx, 0, tags[:, 0]]
    trans_term = trans.astype(np.float64)[tags[:, 1:], tags[:, :-1]]
    em_term = np.take_along_axis(em64[:, 1:], tags[:, 1:, None], axis=2)[..., 0]
    m = mask[:, 1:].astype(np.float64)
    score = score + ((trans_term + em_term) * m).sum(axis=1)
    last_idx = mask.sum(axis=1).astype(np.int64) - 1
    last_tags = np.take_along_axis(tags, last_idx[:, None], axis=1)[:, 0]
    return score + end.astype(np.float64)[last_tags]


def _reference_host(em, tags, mask, trans, start, end):
    """Pure-numpy fp64 fallback (exact semantics incl. arbitrary masks)."""
    em64 = em.astype(np.float64)
    score = start.astype(np.float64) + em64[:, 0]  # [B, T]
    t64 = trans.astype(np.float64)
    for i in range(1, em.shape[1]):
        x = score[:, :, None] + t64[None] + em64[:, i][:, None, :]
        mx = x.max(axis=1)
        nxt = mx + np.log(np.exp(x - mx[:, None, :]).sum(axis=1))
        score = np.where(mask[:, i][:, None], nxt, score)
    x = score + end.astype(np.float64)
    mx = x.max(axis=1, keepdims=True)
    denom = (mx[:, 0] + np.log(np.exp(x - mx).sum(axis=1)))
    numer = _numerator_host(em, tags, mask, trans, start, end)
    return np.float32((denom - numer).mean())


def kernel(**inputs):
    global LAST_RESULTS
    em = np.asarray(inputs["emissions"], dtype=np.float32)
    tags = np.asarray(inputs["tags"])
    mask = np.asarray(inputs["mask"])
    trans = np.asarray(inputs["transitions"], dtype=np.float32)
    start = np.asarray(inputs["start_transitions"], dtype=np.float32)
    end = np.asarray(inputs["end_transitions"], dtype=np.float32)

    if not mask.all():
        # device scan assumes a dense mask (guaranteed by the input spec);
        # fall back to the exact host path otherwise
        return _reference_host(em, tags, mask, trans, start, end)

    _ensure_ntff_hook_importable()
    from concourse.bass_utils import run_bass_kernel_spmd

    nc = _get_nc()
    cs = _kappa_host(em, trans, start) - 1.5   # keep fp8 multipliers ~O(1)
    bf = ml_dtypes.bfloat16
    f8 = ml_dtypes.float8_e4m3fn

    # exp-space, rescaled emission multipliers, [T, S, B] fp8
    ex = np.exp(em.transpose(2, 1, 0) - np.float32(cs)).astype(f8)
    ones_tb = np.ones((T, B), dtype=f8)
    a_exp = np.exp(trans).astype(bf)
    at_exp = np.ascontiguousarray(a_exp.T)
    id_t = np.eye(T, dtype=bf)
    u0 = np.ascontiguousarray(
        np.exp(start[None, :] + em[:, 0, :]).T.astype(f8))      # [T, B]
    endv = np.ascontiguousarray(
        np.tile(np.exp(end).astype(f8)[:, None], (1, B)))

    def emtile(spec):
        kind = spec[0]
        if kind == 'ones':
            return ones_tb[:, None, :]
        if kind == 'u0':
            return u0[:, None, :]
        if kind == 'end':
            return endv[:, None, :]
        return ex[:, spec[1]:spec[1] + 1, :]

    def gather(slots):
        return np.ascontiguousarray(
            np.concatenate([emtile(sp) for _, sp in slots], axis=1))

    in_maps = []
    spans_r = np.zeros(NCORES)
    spans_n = np.zeros(NCORES)
    for c in range(NCORES):
        f, b = _plans(c)
        spans_r[c] = (sum(st != 'I' for st, _ in f)
                      + sum(st != 'I' for st, _ in b))
        spans_n[c] = sum(st != 'I' for st, _ in f[:TAPK + 1])
        fx = id_t if c in (0, 7) else a_exp
        bx = id_t if c in (0, 7) else at_exp
        in_maps.append({
            "emf": gather(f),
            "emb": gather(b),
            "bootf": np.ascontiguousarray(np.concatenate([fx, a_exp], axis=1)),
            "bootb": np.ascontiguousarray(np.concatenate([bx, at_exp], axis=1)),
        })

    LAST_RESULTS = run_bass_kernel_spmd(nc, in_maps, list(range(NCORES)))
    r = np.stack([LAST_RESULTS.results[c]["outr"] for c in range(NCORES)])
    n_tap = np.stack([LAST_RESULTS.results[c]["outn"] for c in range(NCORES)])
    r = r.astype(np.float64).sum(axis=1)    # [8, B]
    n = n_tap.astype(np.float64).sum(axis=1)

    ok = np.isfinite(r).all() and (r > 0).all()
    ok = ok and np.isfinite(n[1:]).all() and (n[1:] > 0).all()
    if not ok:
        return _reference_host(em, tags, mask, trans, start, end)

    logz = (np.log(r).sum(axis=0) - np.log(n[1:]).sum(axis=0)
            + cs * (spans_r.sum() - spans_n[1:].sum()))
    numer = _numerator_host(em, tags, mask, trans, start, end)
    return np.float32((logz - numer).mean())


# revision 34
# speedup vs baseline: 1.0963x; 1.0327x over previous
"""CRF loss kernel for Trainium2 (8 NeuronCores, sequence-parallel).

reference: mean_b( logZ_b - score_b ) for a linear-chain CRF with
B=256, S=512, T=128.

The forward algorithm's 511-step recurrence u_s = (A^T u_{s-1}) o e_s
is latency-bound on device (~0.9 us per step of matmul + DVE multiply
incl. PSUM drain/ack and semaphores), so instead of data-parallel batch
sharding this kernel shards the SEQUENCE: products of positive matrices
forget their starting direction at ~e^-1.9/step (measured), so each
core computes one ~64-step segment for ALL 256 batches, warm-starting
its incoming state with a 2-step warmup, and the host telescopes

    logZ_b = sum_c log r_cb - sum_{c>=1} log n_cb + 511*cs

where r_c = 1^T M_seg_c v~_c (bilinear segment value, computed
meet-in-the-middle with an exact ones/end-vector backward chain),
n_c = 1^T v~_c normalizes the warmup state at the segment boundary, and
cs is a per-step log rescale (from an exact host fp64 forward of batch
0) that keeps everything in range. Serial depth per core: 33 rounds (vs
256 for batch-parallel meet-in-the-middle), each round = fwd step + bwd
step at width 256, anti-phased so TensorE and VectorE ping-pong.

Emissions are pre-exponentiated on the host and shipped as fp8 e4m3
multipliers (64 KB per round vs the ~138 B/ns single-queue DMA feed —
bf16 would be feed-bound; fp8 costs ~2e-1 absolute on a ~3000 logZ,
1e-4 relative). Both chains start from an on-device memset ones state;
per-core inits (u_0 for core 0, exp(end) for core 7) and per-core
segment-length pads ride in the slot-0 emissions with per-core identity
stationaries, so a single uniform program runs on all 8 cores.

Numerator (score of the tagged path, ~0.1% of FLOPs) on host in fp64.
"""

import numpy as np
import ml_dtypes

B, S, T = 256, 512, 128
NCORES = 8
NF = NB = 33               # fwd/bwd slots per core
TAPK = 1                   # n_c tap after fwd slot index 1

_nc_cache = None
LAST_RESULTS = None


def _plans(c):
    """Per-core fwd/bwd slot lists: (stationary, emission_spec).

    stationary: 'A' (=exp(trans), fwd), 'T' (=exp(trans).T, bwd),
    'I' (identity pad). emission_spec: ('step', s) | ('ones',) |
    ('u0',) | ('end',).
    Segments: core 0: steps 1..64, cores 1..6: 64c+1..64c+64,
    core 7: 449..511. Slot 0 doubles as the init loader: with an
    identity stationary and a ones init state, u after slot 0 equals
    the slot-0 "emission" tile.
    """
    if c == 0:
        f = [('I', ('u0',))] + [('A', ('step', s)) for s in range(1, 33)]
        b = [('I', ('ones',))] + [('T', ('step', s))
                                  for s in range(64, 32, -1)]
    elif c < 7:
        f = ([('A', ('step', 64 * c - 1)), ('A', ('step', 64 * c))]
             + [('A', ('step', s)) for s in range(64 * c + 1, 64 * c + 32)])
        b = [('T', ('step', s)) for s in range(64 * c + 64, 64 * c + 31, -1)]
    else:
        f = ([('I', ('ones',)), ('A', ('step', 448))]
             + [('A', ('step', s)) for s in range(449, 480)])
        b = [('I', ('end',))] + [('T', ('step', s))
                                 for s in range(511, 479, -1)]
    assert len(f) == NF and len(b) == NB
    return f, b


def _build_nc():
    import concourse.bacc as bacc
    import concourse.mybir as mybir
    import concourse.tile as tile

    fp32 = mybir.dt.float32
    bf16 = mybir.dt.bfloat16
    f8 = mybir.dt.float8e4
    mult = mybir.AluOpType.mult

    nc = bacc.Bacc("TRN2", target_bir_lowering=False, debug=False)

    emf = nc.dram_tensor("emf", [T, NF, B], f8, kind="ExternalInput")
    emb = nc.dram_tensor("emb", [T, NB, B], f8, kind="ExternalInput")
    bootf = nc.dram_tensor("bootf", [T, 2 * T], bf16, kind="ExternalInput")
    bootb = nc.dram_tensor("bootb", [T, 2 * T], bf16, kind="ExternalInput")
    outr = nc.dram_tensor("outr", [T, B], bf16, kind="ExternalOutput")
    outn = nc.dram_tensor("outn", [T, B], bf16, kind="ExternalOutput")

    chunks = [(0, 2), (2, 4), (6, 6), (12, 7), (19, 7), (26, 7)]

    with tile.TileContext(nc) as tc:
        with (
            tc.tile_pool(name="const", bufs=1) as constp,
            tc.tile_pool(name="emp", bufs=2 * len(chunks)) as emp,
            tc.tile_pool(name="sbp", bufs=4) as sbp,
            tc.tile_pool(name="vp", bufs=2, space="PSUM") as vp,
            tc.tile_pool(name="bp", bufs=2, space="PSUM") as bp,
        ):
            bf_tile = constp.tile([T, 2 * T], bf16)
            bb_tile = constp.tile([T, 2 * T], bf16)
            ones_sb = constp.tile([T, B], bf16)
            nc.gpsimd.memset(ones_sb[:], 1.0)

            emf_tiles = {}
            emb_tiles = {}

            def load_chunk(ci, which="fb"):
                s0, ln = chunks[ci]
                if "f" in which:
                    tf = emp.tile([T, ln, B], f8, tag="emf")
                    nc.sync.dma_start(tf[:], emf[:, s0:s0 + ln, :])
                    emf_tiles[ci] = tf
                if "b" in which:
                    tb = emp.tile([T, ln, B], f8, tag="emb")
                    eng = nc.scalar if which == "b0" else nc.sync
                    eng.dma_start(tb[:], emb[:, s0:s0 + ln, :])
                    emb_tiles[ci] = tb

            def em_slice(tiles, k):
                for ci, (s0, ln) in enumerate(chunks):
                    if s0 <= k < s0 + ln:
                        return tiles[ci][:, k - s0, :]
                raise AssertionError(k)

            # start: fwd boot+lead on the Sync queue, bwd pair in parallel
            # on the Scalar queue; bulk feed stays on Sync (balanced)
            nc.sync.dma_start(bf_tile[:], bootf[:])
            nc.scalar.dma_start(bb_tile[:], bootb[:])
            load_chunk(0, "f")
            load_chunk(0, "b0")
            for ci in range(1, len(chunks)):
                load_chunk(ci)

            fx_ap = bf_tile[:, 0:T]
            fa_ap = bf_tile[:, T:2 * T]
            bx_ap = bb_tile[:, 0:T]
            bat_ap = bb_tile[:, T:2 * T]

            u = ones_sb[:]
            z_prev = ones_sb[:]
            zp = None
            for k in range(NF):
                # fwd slot k: u <- (stat^T u) o e_f[k]
                vf = vp.tile([T, B], fp32, tag="vf")
                nc.tensor.matmul(vf[:], fx_ap if k == 0 else fa_ap, u,
                                 start=True, stop=True)
                u_new = sbp.tile([T, B], bf16, tag="u")
                nc.vector.tensor_tensor(u_new[:], vf[:],
                                        em_slice(emf_tiles, k), mult)
                u = u_new[:]
                # bwd slot k: z <- stat^T (e_b[k] o z)
                tmp = sbp.tile([T, B], bf16, tag="w")
                zsrc = z_prev if zp is None else zp[:]
                nc.vector.tensor_tensor(tmp[:], zsrc,
                                        em_slice(emb_tiles, k), mult)
                zp = bp.tile([T, B], fp32, tag="vb")
                nc.tensor.matmul(zp[:], bx_ap if k == 0 else bat_ap, tmp[:],
                                 start=True, stop=True)
                if k == TAPK:
                    # n_c tap: ship the post-warmup state, host reduces
                    nc.gpsimd.dma_start(outn[:], u)

            # meet: r_c = sum_t z[t,b] * u[t,b]; host does the t-reduction
            prod = sbp.tile([T, B], bf16, tag="u")
            nc.vector.tensor_tensor(prod[:], zp[:], u, mult)
            nc.sync.dma_start(outr[:], prod[:])

    nc.compile()
    return nc


def _get_nc():
    global _nc_cache
    if _nc_cache is None:
        _nc_cache = _build_nc()
    return _nc_cache


def _ensure_ntff_hook_importable():
    """bass_utils imports antenv.axon_hooks when BASS_TRACE is set; this
    image's antenv package lacks that module, so provide a shim rather
    than crash (and enable profiling when the axon .so supports it)."""
    import sys
    import types
    try:
        import antenv.axon_hooks  # noqa: F401
        return
    except ImportError:
        pass
    try:
        import antenv
        from trn_agent_boot.trn_boot import _ntff_profile_via_ctypes
        hook = _ntff_profile_via_ctypes('/opt/axon/libaxon_pjrt.so')
    except Exception:
        try:
            import antenv
        except ImportError:
            return
        hook = None
    mod = types.ModuleType("antenv.axon_hooks")
    mod._hook = hook
    mod.get_axon_ntff_profile_hook = lambda: mod._hook
    mod.set_axon_ntff_profile_hook = lambda h: setattr(mod, "_hook", h)
    antenv.axon_hooks = mod
    sys.modules["antenv.axon_hooks"] = mod


def _kappa_host(em, trans, start):
    """Exact per-step log-mass growth of batch 0 (fp64 log-space forward)."""
    sc = start.astype(np.float64) + em[0, 0].astype(np.float64)
    t64 = trans.astype(np.float64)
    for i in range(1, em.shape[1]):
        x = sc[:, None] + t64 + em[0, i].astype(np.float64)[None, :]
        mx = x.max(axis=0)
        sc = mx + np.log(np.exp(x - mx[None, :]).sum(axis=0))
    mx = sc.max()
    return float((mx + np.log(np.exp(sc - mx).sum())) / (em.shape[1] - 1))


def _numerator_host(em, tags, mask, trans, start, end):
    em64 = em.astype(np.float64)
    tags = tags.astype(np.int64)
    bidx = np.arange(em.shape[0])
    score = start.astype(np.float64)[tags[:, 0]] + em64[bidx, 0, tags[:, 0]]
    trans_term = trans.astype(np.float64)[tags[:, 1:], tags[:, :-1]]
    em_term = np.take_along_axis(em64[:, 1:], tags[:, 1:, None], axis=2)[..., 0]
    m = mask[:, 1:].astype(np.float64)
    score = score + ((trans_term + em_term) * m).sum(axis=1)
    last_idx = mask.sum(axis=1).astype(np.int64) - 1
    last_tags = np.take_along_axis(tags, last_idx[:, None], axis=1)[:, 0]
    return score + end.astype(np.float64)[last_tags]


def _reference_host(em, tags, mask, trans, start, end):
    """Pure-numpy fp64 fallback (exact semantics incl. arbitrary masks)."""
    em64 = em.astype(np.float64)
    score = start.astype(np.float64) + em64[:, 0]  # [B, T]
    t64 = trans.astype(np.float64)
    for i in range(1, em.shape[1]):
        x = score[:, :, None] + t64[None] + em64[:, i][:, None, :]
        mx = x.max(axis=1)
        nxt = mx + np.log(np.exp(x - mx[:, None, :]).sum(axis=1))
        score = np.where(mask[:, i][:, None], nxt, score)
    x = score + end.astype(np.float64)
    mx = x.max(axis=1, keepdims=True)
    denom = (mx[:, 0] + np.log(np.exp(x - mx).sum(axis=1)))
    numer = _numerator_host(em, tags, mask, trans, start, end)
    return np.float32((denom - numer).mean())


def kernel(**inputs):
    global LAST_RESULTS
    em = np.asarray(inputs["emissions"], dtype=np.float32)
    tags = np.asarray(inputs["tags"])
    mask = np.asarray(inputs["mask"])
    trans = np.asarray(inputs["transitions"], dtype=np.float32)
    start = np.asarray(inputs["start_transitions"], dtype=np.float32)
    end = np.asarray(inputs["end_transitions"], dtype=np.float32)

    if not mask.all():
        # device scan assumes a dense mask (guaranteed by the input spec);
        # fall back to the exact host path otherwise
        return _reference_host(em, tags, mask, trans, start, end)

    _ensure_ntff_hook_importable()
    from concourse.bass_utils import run_bass_kernel_spmd

    nc = _get_nc()
    cs = _kappa_host(em, trans, start) - 1.5   # keep fp8 multipliers ~O(1)
    bf = ml_dtypes.bfloat16
    f8 = ml_dtypes.float8_e4m3fn

    # exp-space, rescaled emission multipliers, [T, S, B] fp8
    ex = np.exp(em.transpose(2, 1, 0) - np.float32(cs)).astype(f8)
    ones_tb = np.ones((T, B), dtype=f8)
    a_exp = np.exp(trans).astype(bf)
    at_exp = np.ascontiguousarray(a_exp.T)
    id_t = np.eye(T, dtype=bf)
    u0 = np.ascontiguousarray(
        np.exp(start[None, :] + em[:, 0, :]).T.astype(f8))      # [T, B]
    endv = np.ascontiguousarray(
        np.tile(np.exp(end).astype(f8)[:, None], (1, B)))

    def emtile(spec):
        kind = spec[0]
        if kind == 'ones':
            return ones_tb[:, None, :]
        if kind == 'u0':
            return u0[:, None, :]
        if kind == 'end':
            return endv[:, None, :]
        return ex[:, spec[1]:spec[1] + 1, :]

    def gather(slots):
        return np.ascontiguousarray(
            np.concatenate([emtile(sp) for _, sp in slots], axis=1))

    in_maps = []
    spans_r = np.zeros(NCORES)
    spans_n = np.zeros(NCORES)
    for c in range(NCORES):
        f, b = _plans(c)
        spans_r[c] = (sum(st != 'I' for st, _ in f)
                      + sum(st != 'I' for st, _ in b))
        spans_n[c] = sum(st != 'I' for st, _ in f[:TAPK + 1])
        fx = id_t if c in (0, 7) else a_exp
        bx = id_t if c in (0, 7) else at_exp
        in_maps.append({
            "emf": gather(f),
            "emb": gather(b),
            "bootf": np.ascontiguousarray(np.concatenate([fx, a_exp], axis=1)),
            "bootb": np.ascontiguousarray(np.concatenate([bx, at_exp], axis=1)),
        })

    LAST_RESULTS = run_bass_kernel_spmd(nc, in_maps, list(range(NCORES)))
    r = np.stack([LAST_RESULTS.results[c]["outr"] for c in range(NCORES)])
    n_tap = np.stack([LAST_RESULTS.results[c]["outn"] for c in range(NCORES)])
    r = r.astype(np.float64).sum(axis=1)    # [8, B]
    n = n_tap.astype(np.float64).sum(axis=1)

    ok = np.isfinite(r).all() and (r > 0).all()
    ok = ok and np.isfinite(n[1:]).all() and (n[1:] > 0).all()
    if not ok:
        return _reference_host(em, tags, mask, trans, start, end)

    logz = (np.log(r).sum(axis=0) - np.log(n[1:]).sum(axis=0)
            + cs * (spans_r.sum() - spans_n[1:].sum()))
    numer = _numerator_host(em, tags, mask, trans, start, end)
    return np.float32((logz - numer).mean())
